# revision 1
# baseline (speedup 1.0000x reference)
"""Trainium2 Bass kernel for the ABNet 10-head MLP ensemble + dCBF QP problem.

Sharding: pure data-parallel over the batch axis (B=16384 -> 2048 per core,
8 cores). All per-sample math, including the closed-form 1-constraint QP, is
local to a core; weights are replicated; no collectives.

Per-core compute layout (feature-major, batch in the free dimension):
  xT   [4, BL]        x transposed  (moving operand of layer 1)
  h1   [2048, BL]     = relu(W1.T x) stored as 16 chunks [128, BL] bf16
  L2   x2b[e,b]       = relu(sum_d W2b[d,e] h1[d,b]) via PE, psum [128, 512]
  L3   z3b[c,b]       = sum_e W3b[e,c] x2b[e,b], accumulated in psum at
                        partition offset 32*bt (PE array tiling)
  QP epilogue on DVE/ACT in fp32 on [1, 512] rows, weighted head sum.

Matmuls run in bf16 (1 cycle/row on PE vs 4 for fp32) with fp32 PSUM
accumulation; all non-matmul math stays fp32.
"""

import numpy as np

import concourse.bass as bass
import concourse.bacc as bacc
import concourse.mybir as mybir
from concourse.tile import TileContext
from concourse.bass_utils import run_bass_kernel_spmd
from concourse.masks import make_identity

F32 = mybir.dt.float32
BF16 = mybir.dt.bfloat16
AF = mybir.ActivationFunctionType
ALU = mybir.AluOpType
AX = mybir.AxisListType

OBS_X, OBS_Y, RADIUS = 40.0, 15.0, 6.0
PI = float(np.pi)
TWO_PI = 2.0 * PI

N_CORES = 8
H_FULL, B_FULL, F_FULL, D_FULL, C_FULL = 10, 16384, 4, 2048, 2
BL_FULL = B_FULL // N_CORES

P = 128


def build_nc(H=H_FULL, F=F_FULL, D=D_FULL, C=C_FULL, BL=BL_FULL, NT=512):
    """Build the single-core Bass graph (SPMD: same graph on all cores)."""
    ND = D // P          # contraction chunks (layer 2)
    NE = D // P          # output-feature chunks (layer 2) == L3 contraction
    NB = BL // NT        # batch tiles
    Q = BL // P          # grid columns (sample b = q*128 + p)
    assert D % P == 0 and BL % NT == 0 and NB <= 4 and BL % P == 0

    nc = bacc.Bacc(None, target_bir_lowering=False)

    x_e = nc.declare_dram_parameter("x", [BL, F], F32, isOutput=False)
    W1_e = nc.declare_dram_parameter("W1", [H, F, D], BF16, isOutput=False)
    b1_e = nc.declare_dram_parameter("b1", [H, D], F32, isOutput=False)
    W21_e = nc.declare_dram_parameter("W21", [H, D, D], BF16, isOutput=False)
    b21_e = nc.declare_dram_parameter("b21", [H, D], F32, isOutput=False)
    W22_e = nc.declare_dram_parameter("W22", [H, D, D], BF16, isOutput=False)
    b22_e = nc.declare_dram_parameter("b22", [H, D], F32, isOutput=False)
    W31_e = nc.declare_dram_parameter("W31", [H, D, C], BF16, isOutput=False)
    b31_e = nc.declare_dram_parameter("b31", [H, C], F32, isOutput=False)
    W32_e = nc.declare_dram_parameter("W32", [H, D, C], BF16, isOutput=False)
    b32_e = nc.declare_dram_parameter("b32", [H, C], F32, isOutput=False)
    wt_e = nc.declare_dram_parameter("wt", [H], F32, isOutput=False)
    mean_e = nc.declare_dram_parameter("mean", [F], F32, isOutput=False)
    std_e = nc.declare_dram_parameter("std", [F], F32, isOutput=False)
    out_e = nc.declare_dram_parameter("out", [BL, C], F32, isOutput=True)

    with (
        TileContext(nc) as tc,
        tc.tile_pool(name="cp", bufs=1) as cp,
        tc.tile_pool(name="ps", bufs=4, space="PSUM") as psp,
        tc.tile_pool(name="accp", bufs=2, space="PSUM") as accp,
    ):
        # persistent per-sample rows + small constants
        def crow(tagname):
            return cp.tile([1, BL], F32, tag=tagname, name=tagname)

        xTb = cp.tile([F, BL], BF16, tag="xTb", name="xTb")
        bar16, bdot4, Lf2b = crow("bar16"), crow("bdot4"), crow("Lf2b")
        G0, G1, invGG = crow("G0"), crow("G1"), crow("invGG")
        outacc0, outacc1 = crow("outacc0"), crow("outacc1")
        wrow = cp.tile([1, H], F32, tag="wrow", name="wrow")

        # identity for PE transposes
        ident = cp.tile([P, P], F32, tag="ident", name="ident")
        make_identity(nc, ident)

        # ~100us of light serial DVE work before anything that gates the
        # dense phase: starting the kernel at full blast latches the chip
        # into the 2.0 GHz power state; a gentle ramp keeps it at 2.4.
        warm = cp.tile([1, BL], F32, tag="warm", name="warm")
        nc.vector.memset(warm, 0.0)
        for _ in range(48):
            nc.vector.tensor_scalar(warm, warm, 1.0, None, op0=ALU.add)
        # gate: dummy write into xTb (immediately overwritten by the real
        # producer; exists only to order the dense phase after the ramp)
        nc.vector.tensor_copy(xTb[0:1, 0:1], warm[0:1, 0:1])

        # ------------- preamble (scratch pool, freed afterwards) -----------
        # Per-sample math runs partition-parallel on [128, 16] "grid" tiles
        # (sample b = q*128 + p lives at [p, q]); the six QP vectors the
        # epilogue needs are then transposed back to [1, BL] rows via PE.
        with tc.tile_pool(name="pre", bufs=1) as pre:
            xload = pre.tile([P, Q * F], F32, tag="xload", name="xload")
            nc.sync.dma_start(
                out=xload.rearrange("p (q f) -> p q f", f=F),
                in_=x_e.rearrange("(q p) f -> p q f", p=P),
            )
            xg = xload.rearrange("p (q f) -> p f q", f=F)

            # broadcast std/mean to every partition with a ones-matmul
            smR = pre.tile([1, 2 * F], F32, tag="smR", name="smR")
            nc.sync.dma_start(out=smR[:, 0:F], in_=std_e[None, :])
            nc.sync.dma_start(out=smR[:, F:2 * F], in_=mean_e[None, :])
            ones1 = pre.tile([1, P], F32, tag="ones1", name="ones1")
            nc.vector.memset(ones1, 1.0)
            psb = psp.tile([P, 2 * F], F32, tag="mm", name="ps_bcast")
            nc.tensor.matmul(psb, ones1, smR, start=True, stop=True)
            smB = pre.tile([P, 2 * F], F32, tag="smB", name="smB")
            nc.scalar.copy(smB, psb)

            def grid(nm):
                return pre.tile([P, Q], F32, tag=nm, name=nm)

            x0g = []
            for f in range(F):
                t = grid(f"x0g{f}")
                nc.vector.tensor_scalar(t, xg[:, f, :], smB[:, f:f + 1], None,
                                        op0=ALU.mult)
                nc.vector.tensor_scalar(t, t, smB[:, F + f:F + f + 1], None,
                                        op0=ALU.add)
                x0g.append(t)
            pxg, pyg, thg, vg = x0g

            # sin with range reduction into [-pi, pi] (|arg| < 5*pi)
            def sin_reduced(out_t, arg_ap, sa, sb):
                nc.vector.tensor_scalar(sa, arg_ap, 0.0, None, op0=ALU.add)
                for _ in range(2):
                    nc.vector.tensor_scalar(sb, sa, PI, None, op0=ALU.is_gt)
                    nc.vector.scalar_tensor_tensor(
                        sa, sb, -TWO_PI, sa, op0=ALU.mult, op1=ALU.add
                    )
                    nc.vector.tensor_scalar(sb, sa, -PI, None, op0=ALU.is_lt)
                    nc.vector.scalar_tensor_tensor(
                        sa, sb, TWO_PI, sa, op0=ALU.mult, op1=ALU.add
                    )
                nc.scalar.activation(out_t, sa, AF.Sin)

            sa, sb = grid("sa"), grid("sb")
            st, ct = grid("st"), grid("ct")
            sin_reduced(st, thg, sa, sb)
            thc = grid("thc")
            nc.vector.tensor_scalar(thc, thg, PI / 2.0, None, op0=ALU.add)
            sin_reduced(ct, thc, sa, sb)

            dxg, dyg = grid("dxg"), grid("dyg")
            nc.vector.tensor_scalar(dxg, pxg, -OBS_X, None, op0=ALU.add)
            nc.vector.tensor_scalar(dyg, pyg, -OBS_Y, None, op0=ALU.add)
            vstg, vctg = grid("vstg"), grid("vctg")
            nc.vector.tensor_mul(vstg, vg, st)
            nc.vector.tensor_mul(vctg, vg, ct)

            bar16g, bdot4g, Lf2bg = grid("bar16g"), grid("bdot4g"), grid("Lf2bg")
            G0g, G1g, invGGg = grid("G0g"), grid("G1g"), grid("invGGg")

            # bar16 = 16*(dx^2 + dy^2 - R^2)
            nc.vector.tensor_mul(sa, dxg, dxg)
            nc.vector.tensor_mul(sb, dyg, dyg)
            nc.vector.tensor_add(sa, sa, sb)
            nc.vector.tensor_scalar(
                bar16g, sa, -(RADIUS * RADIUS), 16.0, op0=ALU.add, op1=ALU.mult
            )
            # bdot4 = 8*(dx*vct + dy*vst)
            nc.vector.tensor_mul(sa, dxg, vctg)
            nc.vector.tensor_mul(sb, dyg, vstg)
            nc.vector.tensor_add(sa, sa, sb)
            nc.vector.tensor_scalar(bdot4g, sa, 8.0, None, op0=ALU.mult)
            # Lf2b = 2*v^2
            nc.scalar.activation(Lf2bg, vg, AF.Square, scale=float(np.sqrt(2.0)))
            # G0 = 2*(dx*vst - dy*vct); G1 = -2*(dx*ct + dy*st)
            nc.vector.tensor_mul(sa, dxg, vstg)
            nc.vector.tensor_mul(sb, dyg, vctg)
            nc.vector.tensor_sub(sa, sa, sb)
            nc.vector.tensor_scalar(G0g, sa, 2.0, None, op0=ALU.mult)
            nc.vector.tensor_mul(sa, dxg, ct)
            nc.vector.tensor_mul(sb, dyg, st)
            nc.vector.tensor_add(sa, sa, sb)
            nc.vector.tensor_scalar(G1g, sa, -2.0, None, op0=ALU.mult)
            nc.vector.tensor_mul(sa, G0g, G0g)
            nc.vector.tensor_mul(sb, G1g, G1g)
            nc.vector.tensor_add(sa, sa, sb)
            nc.vector.reciprocal(invGGg, sa)

            # convert grids -> [1, BL] rows (PE transpose + sbuf-sbuf DMA),
            # and the raw x grids -> xTb rows
            def grid_to_row(gt, row_ap, dtype, nm):
                tp = psp.tile([Q, P], F32, tag="mm", name=f"tp_{nm}")
                nc.tensor.matmul(tp, gt, ident, is_transpose=True,
                                 start=True, stop=True)
                cvt = pre.tile([Q, P], dtype, tag="cvt" + dtype.name,
                               name=f"cvt_{nm}", bufs=2)
                nc.scalar.copy(cvt, tp)
                nc.sync.dma_start(
                    out=row_ap.rearrange("one (q p) -> one q p", p=P),
                    in_=cvt,
                )

            for f in range(F):
                grid_to_row(xg[:, f, :], xTb[f:f + 1, :], BF16, f"xtb{f}")
            grid_to_row(bar16g, bar16, F32, "bar16")
            grid_to_row(bdot4g, bdot4, F32, "bdot4")
            grid_to_row(Lf2bg, Lf2b, F32, "lf2b")
            grid_to_row(G0g, G0, F32, "g0")
            grid_to_row(G1g, G1, F32, "g1")
            grid_to_row(invGGg, invGG, F32, "invgg")

            # softmax over wt -> wrow [1, H]
            wt_row = pre.tile([1, H], F32, tag="wt_row", name="wt_row")
            nc.sync.dma_start(out=wt_row, in_=wt_e[None, :])
            wred = pre.tile([1, 1], F32, tag="wred", name="wred")
            nc.vector.reduce_max(wred, wt_row, axis=AX.X)
            nwmax = pre.tile([1, 1], F32, tag="nwmax", name="nwmax")
            nc.vector.tensor_scalar(nwmax, wred, -1.0, None, op0=ALU.mult)
            wexp = pre.tile([1, H], F32, tag="wexp", name="wexp")
            nc.scalar.activation(wexp, wt_row, AF.Exp, bias=nwmax)
            nc.vector.reduce_sum(wred, wexp, axis=AX.X)
            winv = pre.tile([1, 1], F32, tag="winv", name="winv")
            nc.vector.reciprocal(winv, wred)
            nc.vector.tensor_scalar(wrow, wexp, winv, None, op0=ALU.mult)

            nc.vector.memset(outacc0, 0.0)
            nc.vector.memset(outacc1, 0.0)

        # ------------- main pools + head loop ------------------------------
        with (
            tc.tile_pool(name="hw", bufs=2) as hp,      # per-head small tensors
            tc.tile_pool(name="wb", bufs=4) as wbp,     # bf16 weight blocks
            tc.tile_pool(name="h1p", bufs=1) as h1p,
            tc.tile_pool(name="xap", bufs=5) as xap,
            tc.tile_pool(name="ep", bufs=8) as ep,     # epilogue scratch
        ):
            zNT = cp.tile([P, NT], BF16, tag="zNT", name="zNT")
            nc.vector.memset(zNT, 0.0)

            pending_epi = []

            for h in range(H):
                # per-head small tensors
                w1tb = hp.tile([F, D], BF16, tag="w1tb", name=f"w1tb_{h}")
                nc.sync.dma_start(out=w1tb, in_=W1_e[h])

                b1t = hp.tile([P, ND], F32, tag="b1t", name=f"b1t_{h}")
                nc.sync.dma_start(
                    out=b1t, in_=b1_e[h].rearrange("(dc p) -> p dc", p=P)
                )
                b21t = hp.tile([P, NE], F32, tag="b21t", name=f"b21t_{h}")
                nc.sync.dma_start(
                    out=b21t, in_=b21_e[h].rearrange("(ec p) -> p ec", p=P)
                )
                b22t = hp.tile([P, NE], F32, tag="b22t", name=f"b22t_{h}")
                nc.sync.dma_start(
                    out=b22t, in_=b22_e[h].rearrange("(ec p) -> p ec", p=P)
                )

                w31t = hp.tile([P, NE * C], BF16, tag="w31t", name=f"w31t_{h}")
                nc.sync.dma_start(
                    out=w31t.rearrange("p (ec c) -> p ec c", c=C),
                    in_=W31_e[h].rearrange("(ec p) c -> p ec c", p=P),
                )
                w32t = hp.tile([P, NE * C], BF16, tag="w32t", name=f"w32t_{h}")
                nc.sync.dma_start(
                    out=w32t.rearrange("p (ec c) -> p ec c", c=C),
                    in_=W32_e[h].rearrange("(ec p) c -> p ec c", p=P),
                )

                b31R = hp.tile([1, C], F32, tag="b31R", name=f"b31R_{h}")
                nc.sync.dma_start(out=b31R, in_=b31_e[h][None, :])
                b32R = hp.tile([1, C], F32, tag="b32R", name=f"b32R_{h}")
                nc.sync.dma_start(out=b32R, in_=b32_e[h][None, :])

                # ---- layers 2+3 ----
                # Per branch one psum bank; batch-tile accumulators live at
                # partition offsets 0/32/64/96 (PE-array tile positions).
                acc31 = accp.tile([128, NT], F32, tag="acc31", name=f"acc31_{h}")
                acc32 = accp.tile([128, NT], F32, tag="acc32", name=f"acc32_{h}")
                accs = (acc31, acc32)

                def load_wblock(W_e, e, nm, cast_engine):
                    wb = wbp.tile([P, D], BF16, tag="wb", name=f"wb{nm}_{h}_{e}")
                    nc.sync.dma_start(
                        out=wb.rearrange("p (dc j) -> p dc j", j=P),
                        in_=W_e[h][:, e * P:(e + 1) * P].rearrange(
                            "(dc p) j -> p dc j", p=P
                        ),
                    )
                    return wb

                # L3 matmuls are deferred by one L2 group so they never stall
                # on their activation; pending holds at most one closure.
                pending_l3 = []

                def flush_l3():
                    while pending_l3:
                        pending_l3.pop(0)()

                def l2_group(e, br, bt, wb, b2t, w3t):
                    ps2 = psp.tile(
                        [P, NT], F32, tag="mm", name=f"ps2_{h}_{e}_{br}_{bt}"
                    )
                    for dc in range(ND):
                        nc.tensor.matmul(
                            ps2,
                            wb[:, dc * P:(dc + 1) * P],
                            h1[dc][:, bt * NT:(bt + 1) * NT],
                            start=(dc == 0),
                            stop=(dc == ND - 1),
                        )
                    flush_l3()
                    xa = xap.tile(
                        [P, NT], BF16, tag="xa", name=f"xa_{h}_{e}_{br}_{bt}"
                    )
                    nc.scalar.activation(xa, ps2, AF.Relu, bias=b2t[:, e:e + 1])
                    sl = 32 * bt

                    def emit_l3():
                        nc.tensor.matmul(
                            accs[br][sl:sl + 2, :],
                            w3t[:, C * e:C * (e + 1)],
                            xa,
                            start=(e == 0),
                            stop=(e == NE - 1),
                            skip_group_check=True,
                            tile_position=(0, sl),
                        )

                    pending_l3.append(emit_l3)

                # ---- layer 1 interleaved with L2(e=0) ----
                # h1 is produced batch-tile by batch-tile; as soon as a batch
                # tile is complete, the e=0/br=0 L2 group for it runs. This
                # keeps PE dense across the head boundary (no HAM throttle).
                wb21_0 = load_wblock(W21_e, 0, "21", "v")
                wb22_0 = load_wblock(W22_e, 0, "22", "s")

                h1 = [
                    h1p.tile([P, BL], BF16, tag=f"h1_{dc}", name=f"h1_{h}_{dc}")
                    for dc in range(ND)
                ]
                for bt in range(NB):
                    for dc in range(ND):
                        ps1 = psp.tile([P, NT], F32, tag="mm",
                                       name=f"ps1_{h}_{dc}_{bt}")
                        nc.tensor.matmul(
                            ps1,
                            w1tb[:, dc * P:(dc + 1) * P],
                            xTb[:, bt * NT:(bt + 1) * NT],
                            start=True,
                            stop=True,
                        )
                        h1s = h1[dc][:, bt * NT:(bt + 1) * NT]
                        if dc % 2 == 0:
                            nc.scalar.activation(
                                h1s, ps1, AF.Relu, bias=b1t[:, dc:dc + 1]
                            )
                        else:
                            # relu(z+b) on DVE: (z + bias) max zeros
                            nc.vector.scalar_tensor_tensor(
                                h1s, ps1, b1t[:, dc:dc + 1], zNT,
                                op0=ALU.add, op1=ALU.max,
                            )
                    l2_group(0, 0, bt, wb21_0, b21t, w31t)
                # previous head's QP epilogue lands here: after this head's
                # L1 acts are queued, so the act engines never stall PE at
                # the head boundary.
                while pending_epi:
                    pending_epi.pop(0)()
                for bt in range(NB):
                    l2_group(0, 1, bt, wb22_0, b22t, w32t)

                for e in range(1, NE):
                    wb21 = load_wblock(W21_e, e, "21", "v")
                    wb22 = load_wblock(W22_e, e, "22", "s")
                    for br, (wb, b2t, w3t) in enumerate(
                        ((wb21, b21t, w31t), (wb22, b22t, w32t))
                    ):
                        for bt in range(NB):
                            l2_group(e, br, bt, wb, b2t, w3t)
                flush_l3()

                # ---- QP epilogue (deferred into the next head) ----
                def emit_epilogue(h=h, acc31=acc31, acc32=acc32,
                                  b31R=b31R, b32R=b32R):
                  for bt in range(NB):
                    bs = slice(bt * NT, (bt + 1) * NT)
                    sl = 32 * bt

                    def et(tagname):
                        return ep.tile([1, NT], F32, tag="eps",
                                       name=f"{tagname}_{h}_{bt}")

                    # Compute engines can only start at partitions 0/32/64/96,
                    # so copy the [2, NT] psum slice to SBUF (legal, starts at
                    # sl) and DMA row 1 down to a partition-0 tile.
                    t31 = ep.tile([2, NT], F32, tag="t2", name=f"t31_{h}_{bt}", bufs=3)
                    nc.vector.tensor_copy(t31, acc31[sl:sl + 2, :])
                    t32 = ep.tile([2, NT], F32, tag="t2", name=f"t32_{h}_{bt}", bufs=3)
                    nc.vector.tensor_copy(t32, acc32[sl:sl + 2, :])
                    z31_1 = et("z31_1")
                    nc.sync.dma_start(out=z31_1, in_=t31[1:2, :])
                    z32_1 = et("z32_1")
                    nc.sync.dma_start(out=z32_1, in_=t32[1:2, :])

                    s0 = et("s0")
                    nc.scalar.activation(
                        s0, t32[0:1, :], AF.Sigmoid, bias=b32R[:, 0:1]
                    )
                    s1 = et("s1")
                    nc.scalar.activation(s1, z32_1, AF.Sigmoid, bias=b32R[:, 1:2])
                    x31_0 = et("x31_0")
                    nc.vector.tensor_scalar(
                        x31_0, t31[0:1, :], b31R[:, 0:1], None, op0=ALU.add
                    )
                    x31_1 = et("x31_1")
                    nc.vector.tensor_scalar(x31_1, z31_1, b31R[:, 1:2], None, op0=ALU.add)

                    ssum = et("ssum")
                    nc.vector.tensor_add(ssum, s0, s1)
                    sprod = et("sprod")
                    nc.vector.tensor_mul(sprod, s0, s1)

                    # h_rhs = Lf2b + ssum*bdot4 + sprod*bar16
                    nc.vector.tensor_mul(ssum, ssum, bdot4[:, bs])
                    nc.vector.tensor_mul(sprod, sprod, bar16[:, bs])
                    nc.vector.tensor_add(ssum, ssum, sprod)
                    hrhs = et("hrhs")
                    nc.vector.tensor_add(hrhs, ssum, Lf2b[:, bs])

                    # Gu = G0*x31_0 + G1*x31_1
                    gu0 = et("gu0")
                    nc.vector.tensor_mul(gu0, G0[:, bs], x31_0)
                    gu1 = et("gu1")
                    nc.vector.tensor_mul(gu1, G1[:, bs], x31_1)
                    nc.vector.tensor_add(gu0, gu0, gu1)

                    # lam = relu(Gu - hrhs) * invGG
                    nc.vector.tensor_sub(gu0, gu0, hrhs)
                    nc.vector.tensor_scalar_max(gu0, gu0, 0.0)
                    lam = et("lam")
                    nc.vector.tensor_mul(lam, gu0, invGG[:, bs])

                    # u_c = x31_c - lam*G_c ; outacc_c += w[h]*u_c
                    lg0 = et("lg0")
                    nc.vector.tensor_mul(lg0, lam, G0[:, bs])
                    nc.vector.tensor_sub(x31_0, x31_0, lg0)
                    nc.vector.scalar_tensor_tensor(
                        outacc0[:, bs], x31_0, wrow[:, h:h + 1], outacc0[:, bs],
                        op0=ALU.mult, op1=ALU.add,
                    )
                    lg1 = et("lg1")
                    nc.vector.tensor_mul(lg1, lam, G1[:, bs])
                    nc.vector.tensor_sub(x31_1, x31_1, lg1)
                    nc.vector.scalar_tensor_tensor(
                        outacc1[:, bs], x31_1, wrow[:, h:h + 1], outacc1[:, bs],
                        op0=ALU.mult, op1=ALU.add,
                    )

                pending_epi.append(emit_epilogue)

            while pending_epi:
                pending_epi.pop(0)()

            # ---------------- output ---------------------------------------
            # rows -> [128, 16x2] grid via PE transpose, then one near-
            # contiguous DMA (8-byte segments) instead of 4-byte scatters.
            outT = ep.tile([P, Q * C], F32, tag="outT", name="outT", bufs=1)
            for c, row in ((0, outacc0), (1, outacc1)):
                og = ep.tile([Q, P], F32, tag="og", name=f"og_{c}", bufs=2)
                nc.sync.dma_start(
                    out=og, in_=row.rearrange("one (q p) -> one q p", p=P)
                )
                tpo = psp.tile([P, Q], F32, tag="mm", name=f"tpo_{c}")
                nc.tensor.matmul(tpo, og, ident[0:Q, 0:Q], is_transpose=True,
                                 start=True, stop=True)
                nc.scalar.copy(
                    outT.rearrange("p (q c) -> p c q", c=C)[:, c, :], tpo
                )
            nc.sync.dma_start(
                out=out_e.rearrange("(q p) c -> p q c", p=P),
                in_=outT.rearrange("p (q c) -> p q c", c=C),
            )

    nc.finalize()
    return nc


_nc_cache = None


def _get_nc():
    global _nc_cache
    if _nc_cache is None:
        _nc_cache = build_nc()
    return _nc_cache


_WEIGHT_NAMES = (
    "W1", "b1", "W21", "b21", "W22", "b22",
    "W31", "b31", "W32", "b32", "wt", "mean", "std",
)


_BF16_NAMES = ("W1", "W21", "W22", "W31", "W32")


def kernel(**inputs) -> np.ndarray:
    import ml_dtypes

    x = np.ascontiguousarray(np.asarray(inputs["x"], dtype=np.float32))
    rep = {}
    for k in _WEIGHT_NAMES:
        a = np.asarray(inputs[k], dtype=np.float32)
        if k in _BF16_NAMES:
            a = a.astype(ml_dtypes.bfloat16)
        rep[k] = np.ascontiguousarray(a)
    nc = _get_nc()
    in_maps = []
    for i in range(N_CORES):
        m = dict(rep)
        m["x"] = np.ascontiguousarray(x[i * BL_FULL:(i + 1) * BL_FULL])
        in_maps.append(m)
    globals()["_last_in_maps"] = in_maps
    res = run_bass_kernel_spmd(nc, in_maps, core_ids=list(range(N_CORES)))
    outs = [np.asarray(res.results[i]["out"]) for i in range(N_CORES)]
    return np.concatenate(outs, axis=0).astype(np.float32)



# revision 23
# speedup vs baseline: 1.2007x; 1.2007x over previous
"""Trainium2 Bass kernel for the ABNet 10-head MLP ensemble + dCBF QP problem.

Sharding: pure data-parallel over the batch axis (B=16384 -> 2048 per core,
8 cores). All per-sample math, including the closed-form 1-constraint QP, is
local to a core; weights are replicated; no collectives.

Per-core compute layout (feature-major, batch in the free dimension):
  xT   [4, BL]        x transposed  (moving operand of layer 1)
  h1   [2048, BL]     = relu(W1.T x) stored as 16 chunks [128, BL] bf16
  L2   x2b[e,b]       = relu(sum_d W2b[d,e] h1[d,b]) via PE, psum [128, 512]
  L3   z3b[c,b]       = sum_e W3b[e,c] x2b[e,b], accumulated in psum at
                        partition offset 32*bt (PE array tiling)
  QP epilogue on DVE/ACT in fp32 on [1, 512] rows, weighted head sum.

Matmuls run in bf16 (1 cycle/row on PE vs 4 for fp32) with fp32 PSUM
accumulation; all non-matmul math stays fp32.

Branch 2 (W22 -> x22 -> x32 -> sigmoid CBF params) runs in fp8e4 with
DoubleRow perf mode (2 contraction rows per PE cycle, ~1.8x measured):
the sigmoid + QP structure fully absorbs fp8 quantization error
(measured end-to-end rel-err identical to all-bf16). Branch 1 (x31,
the control path) must stay bf16 (fp8 there fails the 2e-2 gate).
Scales: W22/W32 pre-scaled x64 on host; h1/x22 activations x16 on
device; descale folded into the next activation's scale operand.
"""

import numpy as np

import concourse.bass as bass
import concourse.bacc as bacc
import concourse.mybir as mybir
from concourse.tile import TileContext
from concourse.bass_utils import run_bass_kernel_spmd
from concourse.masks import make_identity

F32 = mybir.dt.float32
BF16 = mybir.dt.bfloat16
F8 = mybir.dt.float8e4
DR = mybir.MatmulPerfMode.DoubleRow
AF = mybir.ActivationFunctionType
ALU = mybir.AluOpType
AX = mybir.AxisListType

W8SC = 64.0   # host-side fp8 weight scale (W22, W32)
A8SC = 16.0   # on-device fp8 activation scale (h1, x22)

OBS_X, OBS_Y, RADIUS = 40.0, 15.0, 6.0
PI = float(np.pi)
TWO_PI = 2.0 * PI

N_CORES = 8
H_FULL, B_FULL, F_FULL, D_FULL, C_FULL = 10, 16384, 4, 2048, 2
BL_FULL = B_FULL // N_CORES

P = 128


def build_nc(H=H_FULL, F=F_FULL, D=D_FULL, C=C_FULL, BL=BL_FULL, NT=512):
    """Build the single-core Bass graph (SPMD: same graph on all cores)."""
    ND = D // P          # contraction chunks (layer 2)
    NE = D // P          # output-feature chunks (layer 2) == L3 contraction
    NB = BL // NT        # batch tiles
    Q = BL // P          # grid columns (sample b = q*128 + p)
    assert D % P == 0 and BL % NT == 0 and NB <= 4 and BL % P == 0

    nc = bacc.Bacc(None, target_bir_lowering=False)

    x_e = nc.declare_dram_parameter("x", [BL, F], F32, isOutput=False)
    W1_e = nc.declare_dram_parameter("W1", [H, F, D], BF16, isOutput=False)
    b1_e = nc.declare_dram_parameter("b1", [H, D], F32, isOutput=False)
    W21_e = nc.declare_dram_parameter("W21", [H, D, D], BF16, isOutput=False)
    b21_e = nc.declare_dram_parameter("b21", [H, D], F32, isOutput=False)
    W22_e = nc.declare_dram_parameter("W22", [H, D, D], F8, isOutput=False)
    b22_e = nc.declare_dram_parameter("b22", [H, D], F32, isOutput=False)
    W31_e = nc.declare_dram_parameter("W31", [H, D, C], BF16, isOutput=False)
    b31_e = nc.declare_dram_parameter("b31", [H, C], F32, isOutput=False)
    W32_e = nc.declare_dram_parameter("W32", [H, D, C], F8, isOutput=False)
    b32_e = nc.declare_dram_parameter("b32", [H, C], F32, isOutput=False)
    wt_e = nc.declare_dram_parameter("wt", [H], F32, isOutput=False)
    mean_e = nc.declare_dram_parameter("mean", [F], F32, isOutput=False)
    std_e = nc.declare_dram_parameter("std", [F], F32, isOutput=False)
    out_e = nc.declare_dram_parameter("out", [BL, C], F32, isOutput=True)

    with (
        TileContext(nc) as tc,
        tc.tile_pool(name="cp", bufs=1) as cp,
        tc.tile_pool(name="ps", bufs=4, space="PSUM") as psp,
        tc.tile_pool(name="accp", bufs=2, space="PSUM") as accp,
    ):
        # persistent per-sample rows + small constants
        def crow(tagname):
            return cp.tile([1, BL], F32, tag=tagname, name=tagname)

        xTb = cp.tile([F, BL], BF16, tag="xTb", name="xTb")
        # QP constraint vectors and output accumulators live in GRID form
        # [128, Q] (sample b = q*128 + p at [p, q]) — partition-parallel
        # epilogue math and only 64B/partition each (vs 8KB for [1,BL] rows)
        def cgrid(nm):
            return cp.tile([P, Q], F32, tag=nm, name=nm)

        bar16g, bdot4g, Lf2bg = cgrid("bar16g"), cgrid("bdot4g"), cgrid("Lf2bg")
        G0g, G1g, invGGg = cgrid("G0g"), cgrid("G1g"), cgrid("invGGg")
        outacc0g, outacc1g = cgrid("outacc0g"), cgrid("outacc1g")
        wrow = cp.tile([1, H], F32, tag="wrow", name="wrow")
        # per-head scalars broadcast to all 128 partitions (grid-math biases)
        wB = cp.tile([P, H], F32, tag="wB", name="wB")
        B31B = cp.tile([P, H * C], F32, tag="B31B", name="B31B")
        B32B = cp.tile([P, H * C], F32, tag="B32B", name="B32B")

        # identity for PE transposes
        ident = cp.tile([P, P], F32, tag="ident", name="ident")
        make_identity(nc, ident)

        # ~100us of light serial DVE work before anything that gates the
        # dense phase: starting the kernel at full blast latches the chip
        # into the 2.0 GHz power state; a gentle ramp keeps it at 2.4.
        warm = cp.tile([1, NT], F32, tag="warm", name="warm")
        nc.vector.memset(warm, 0.0)
        for _ in range(192):
            nc.vector.tensor_scalar(warm, warm, 1.0, None, op0=ALU.add)
        # gate: dummy write into xTb (immediately overwritten by the real
        # producer; exists only to order the dense phase after the ramp)
        nc.vector.tensor_copy(xTb[0:1, 0:1], warm[0:1, 0:1])

        # ------------- preamble (scratch pool, freed afterwards) -----------
        # Per-sample math runs partition-parallel on [128, 16] "grid" tiles
        # (sample b = q*128 + p lives at [p, q]); the six QP vectors the
        # epilogue needs are then transposed back to [1, BL] rows via PE.
        with tc.tile_pool(name="pre", bufs=1) as pre:
            xload = pre.tile([P, Q * F], F32, tag="xload", name="xload")
            nc.sync.dma_start(
                out=xload.rearrange("p (q f) -> p q f", f=F),
                in_=x_e.rearrange("(q p) f -> p q f", p=P),
            )
            xg = xload.rearrange("p (q f) -> p f q", f=F)

            # broadcast std/mean to every partition with a ones-matmul
            smR = pre.tile([1, 2 * F], F32, tag="smR", name="smR")
            nc.sync.dma_start(out=smR[:, 0:F], in_=std_e[None, :])
            nc.sync.dma_start(out=smR[:, F:2 * F], in_=mean_e[None, :])
            ones1 = pre.tile([1, P], F32, tag="ones1", name="ones1")
            nc.vector.memset(ones1, 1.0)
            psb = psp.tile([P, 2 * F], F32, tag="mm", name="ps_bcast")
            nc.tensor.matmul(psb, ones1, smR, start=True, stop=True)
            smB = pre.tile([P, 2 * F], F32, tag="smB", name="smB")
            nc.scalar.copy(smB, psb)

            def grid(nm):
                return pre.tile([P, Q], F32, tag=nm, name=nm)

            x0g = []
            for f in range(F):
                t = grid(f"x0g{f}")
                nc.vector.tensor_scalar(t, xg[:, f, :], smB[:, f:f + 1], None,
                                        op0=ALU.mult)
                nc.vector.tensor_scalar(t, t, smB[:, F + f:F + f + 1], None,
                                        op0=ALU.add)
                x0g.append(t)
            pxg, pyg, thg, vg = x0g

            # sin with range reduction into [-pi, pi] (|arg| < 5*pi)
            def sin_reduced(out_t, arg_ap, sa, sb):
                nc.vector.tensor_scalar(sa, arg_ap, 0.0, None, op0=ALU.add)
                for _ in range(2):
                    nc.vector.tensor_scalar(sb, sa, PI, None, op0=ALU.is_gt)
                    nc.vector.scalar_tensor_tensor(
                        sa, sb, -TWO_PI, sa, op0=ALU.mult, op1=ALU.add
                    )
                    nc.vector.tensor_scalar(sb, sa, -PI, None, op0=ALU.is_lt)
                    nc.vector.scalar_tensor_tensor(
                        sa, sb, TWO_PI, sa, op0=ALU.mult, op1=ALU.add
                    )
                nc.scalar.activation(out_t, sa, AF.Sin)

            sa, sb = grid("sa"), grid("sb")
            st, ct = grid("st"), grid("ct")
            sin_reduced(st, thg, sa, sb)
            thc = grid("thc")
            nc.vector.tensor_scalar(thc, thg, PI / 2.0, None, op0=ALU.add)
            sin_reduced(ct, thc, sa, sb)

            dxg, dyg = grid("dxg"), grid("dyg")
            nc.vector.tensor_scalar(dxg, pxg, -OBS_X, None, op0=ALU.add)
            nc.vector.tensor_scalar(dyg, pyg, -OBS_Y, None, op0=ALU.add)
            vstg, vctg = grid("vstg"), grid("vctg")
            nc.vector.tensor_mul(vstg, vg, st)
            nc.vector.tensor_mul(vctg, vg, ct)

            # bar16 = 16*(dx^2 + dy^2 - R^2)
            nc.vector.tensor_mul(sa, dxg, dxg)
            nc.vector.tensor_mul(sb, dyg, dyg)
            nc.vector.tensor_add(sa, sa, sb)
            nc.vector.tensor_scalar(
                bar16g, sa, -(RADIUS * RADIUS), 16.0, op0=ALU.add, op1=ALU.mult
            )
            # bdot4 = 8*(dx*vct + dy*vst)
            nc.vector.tensor_mul(sa, dxg, vctg)
            nc.vector.tensor_mul(sb, dyg, vstg)
            nc.vector.tensor_add(sa, sa, sb)
            nc.vector.tensor_scalar(bdot4g, sa, 8.0, None, op0=ALU.mult)
            # Lf2b = 2*v^2
            nc.scalar.activation(Lf2bg, vg, AF.Square, scale=float(np.sqrt(2.0)))
            # G0 = 2*(dx*vst - dy*vct); G1 = -2*(dx*ct + dy*st)
            nc.vector.tensor_mul(sa, dxg, vstg)
            nc.vector.tensor_mul(sb, dyg, vctg)
            nc.vector.tensor_sub(sa, sa, sb)
            nc.vector.tensor_scalar(G0g, sa, 2.0, None, op0=ALU.mult)
            nc.vector.tensor_mul(sa, dxg, ct)
            nc.vector.tensor_mul(sb, dyg, st)
            nc.vector.tensor_add(sa, sa, sb)
            nc.vector.tensor_scalar(G1g, sa, -2.0, None, op0=ALU.mult)
            nc.vector.tensor_mul(sa, G0g, G0g)
            nc.vector.tensor_mul(sb, G1g, G1g)
            nc.vector.tensor_add(sa, sa, sb)
            nc.vector.reciprocal(invGGg, sa)

            # convert the raw x grids -> xTb rows (PE transpose + DMA)
            def grid_to_row(gt, row_ap, dtype, nm):
                tp = psp.tile([Q, P], F32, tag="mm", name=f"tp_{nm}")
                nc.tensor.matmul(tp, gt, ident, is_transpose=True,
                                 start=True, stop=True)
                cvt = pre.tile([Q, P], dtype, tag="cvt" + dtype.name,
                               name=f"cvt_{nm}", bufs=2)
                nc.scalar.copy(cvt, tp)
                nc.sync.dma_start(
                    out=row_ap.rearrange("one (q p) -> one q p", p=P),
                    in_=cvt,
                )

            for f in range(F):
                grid_to_row(xg[:, f, :], xTb[f:f + 1, :], BF16, f"xtb{f}")

            # broadcast b31/b32 (all heads) to every partition: [P, H*C]
            b3R = pre.tile([1, 2 * H * C], F32, tag="b3R", name="b3R")
            nc.sync.dma_start(
                out=b3R[:, 0:H * C].rearrange("one (h c) -> one h c", c=C),
                in_=b31_e[None, :, :],
            )
            nc.sync.dma_start(
                out=b3R[:, H * C:].rearrange("one (h c) -> one h c", c=C),
                in_=b32_e[None, :, :],
            )
            psb3 = psp.tile([P, 2 * H * C], F32, tag="mm", name="ps_b3")
            nc.tensor.matmul(psb3, ones1, b3R, start=True, stop=True)
            nc.scalar.copy(B31B, psb3[:, 0:H * C])
            nc.scalar.copy(B32B, psb3[:, H * C:])

            # softmax over wt -> wrow [1, H]
            wt_row = pre.tile([1, H], F32, tag="wt_row", name="wt_row")
            nc.sync.dma_start(out=wt_row, in_=wt_e[None, :])
            wred = pre.tile([1, 1], F32, tag="wred", name="wred")
            nc.vector.reduce_max(wred, wt_row, axis=AX.X)
            nwmax = pre.tile([1, 1], F32, tag="nwmax", name="nwmax")
            nc.vector.tensor_scalar(nwmax, wred, -1.0, None, op0=ALU.mult)
            wexp = pre.tile([1, H], F32, tag="wexp", name="wexp")
            nc.scalar.activation(wexp, wt_row, AF.Exp, bias=nwmax)
            nc.vector.reduce_sum(wred, wexp, axis=AX.X)
            winv = pre.tile([1, 1], F32, tag="winv", name="winv")
            nc.vector.reciprocal(winv, wred)
            nc.vector.tensor_scalar(wrow, wexp, winv, None, op0=ALU.mult)
            psw = psp.tile([P, H], F32, tag="mm", name="ps_w")
            nc.tensor.matmul(psw, ones1, wrow, start=True, stop=True)
            nc.scalar.copy(wB, psw)

            nc.vector.memset(outacc0g, 0.0)
            nc.vector.memset(outacc1g, 0.0)

        # ------------- main pools + head loop ------------------------------
        with (
            tc.tile_pool(name="hw", bufs=2) as hp,      # per-head small tensors
            tc.tile_pool(name="wb", bufs=4) as wbp,     # bf16 weight blocks
            tc.tile_pool(name="h1p", bufs=1) as h1p,
            tc.tile_pool(name="xap", bufs=5) as xap,
            tc.tile_pool(name="ep", bufs=8) as ep,     # epilogue scratch
        ):
            zNT = cp.tile([P, NT], BF16, tag="zNT", name="zNT")
            nc.vector.memset(zNT, 0.0)

            pending_epi = []

            for h in range(H):
                # per-head small tensors
                w1tb = hp.tile([F, D], BF16, tag="w1tb", name=f"w1tb_{h}")
                nc.sync.dma_start(out=w1tb, in_=W1_e[h])

                b1t = hp.tile([P, ND], F32, tag="b1t", name=f"b1t_{h}")
                nc.sync.dma_start(
                    out=b1t, in_=b1_e[h].rearrange("(dc p) -> p dc", p=P)
                )
                b21t = hp.tile([P, NE], F32, tag="b21t", name=f"b21t_{h}")
                nc.sync.dma_start(
                    out=b21t, in_=b21_e[h].rearrange("(ec p) -> p ec", p=P)
                )
                b22t = hp.tile([P, NE], F32, tag="b22t", name=f"b22t_{h}")
                nc.sync.dma_start(
                    out=b22t, in_=b22_e[h].rearrange("(ec p) -> p ec", p=P)
                )
                # x16-scaled biases for the fp8 activations:
                # relu(16z + 16b) = 16 relu(z + b)
                b1t16 = hp.tile([P, ND], F32, tag="b1t16", name=f"b1t16_{h}")
                nc.vector.tensor_scalar(b1t16, b1t, A8SC, None, op0=ALU.mult)
                b22t16 = hp.tile([P, NE], F32, tag="b22t16", name=f"b22t16_{h}")
                nc.vector.tensor_scalar(b22t16, b22t, A8SC, None, op0=ALU.mult)

                w31t = hp.tile([P, NE * C], BF16, tag="w31t", name=f"w31t_{h}")
                nc.sync.dma_start(
                    out=w31t.rearrange("p (ec c) -> p ec c", c=C),
                    in_=W31_e[h].rearrange("(ec p) c -> p ec c", p=P),
                )
                w32t = hp.tile([P, NE * C], F8, tag="w32t", name=f"w32t_{h}")
                nc.sync.dma_start(
                    out=w32t.rearrange("p (ec c) -> p ec c", c=C),
                    in_=W32_e[h].rearrange("(ec p) c -> p ec c", p=P),
                )

                # ---- layers 2+3 ----
                # Per branch one psum bank; batch-tile accumulators live at
                # partition offsets 0/32/64/96 (PE-array tile positions).
                acc31 = accp.tile([128, NT], F32, tag="acc31", name=f"acc31_{h}")
                acc32 = accp.tile([128, NT], F32, tag="acc32", name=f"acc32_{h}")
                accs = (acc31, acc32)

                def load_wblock(W_e, e, nm, cast_engine):
                    wb = wbp.tile([P, D], BF16, tag="wb", name=f"wb{nm}_{h}_{e}",
                                  bufs=2)
                    nc.sync.dma_start(
                        out=wb.rearrange("p (dc j) -> p dc j", j=P),
                        in_=W_e[h][:, e * P:(e + 1) * P].rearrange(
                            "(dc p) j -> p dc j", p=P
                        ),
                    )
                    return wb

                def load_wblock8(e):
                    # fp8 W22 e-block in DoubleRow pair layout:
                    # wb[p, dp, i, j] = 64*W22[dp*256 + i*128 + p, e*128 + j]
                    wb = wbp.tile([P, D], F8, tag="wb8", name=f"wb8_{h}_{e}",
                                  bufs=2)
                    nc.sync.dma_start(
                        out=wb.rearrange("p (dp i j) -> p dp i j", i=2, j=P),
                        in_=W22_e[h][:, e * P:(e + 1) * P].rearrange(
                            "(dp i p) j -> p dp i j", i=2, p=P
                        ),
                    )
                    return wb

                # L3 matmuls are deferred by one L2 group so they never stall
                # on their activation; pending holds at most one closure.
                pending_l3 = []

                def flush_l3():
                    while pending_l3:
                        pending_l3.pop(0)()

                def defer_l3(br, e, bt, w3t, xa):
                    sl = 32 * bt

                    def emit_l3():
                        nc.tensor.matmul(
                            accs[br][sl:sl + 2, :],
                            w3t[:, C * e:C * (e + 1)],
                            xa,
                            start=(e == 0),
                            stop=(e == NE - 1),
                            skip_group_check=True,
                            tile_position=(0, sl),
                        )

                    pending_l3.append(emit_l3)

                def l2_group_b1(e, bt, wb):
                    ps2 = psp.tile(
                        [P, NT], F32, tag="mm", name=f"ps2_{h}_{e}_0_{bt}"
                    )
                    for dc in range(ND):
                        nc.tensor.matmul(
                            ps2,
                            wb[:, dc * P:(dc + 1) * P],
                            h1[dc][:, bt * NT:(bt + 1) * NT],
                            start=(dc == 0),
                            stop=(dc == ND - 1),
                        )
                    flush_l3()
                    xa = xap.tile(
                        [P, NT], BF16, tag="xa", name=f"xa_{h}_{e}_0_{bt}"
                    )
                    # relu(z+b) on DVE (ACT is saturated by the fp8 copies)
                    nc.vector.scalar_tensor_tensor(
                        xa, ps2, b21t[:, e:e + 1], zNT, op0=ALU.add, op1=ALU.max
                    )
                    defer_l3(0, e, bt, w31t, xa)

                def l2_group_b2(e, bt, wb8):
                    ps2 = psp.tile(
                        [P, NT], F32, tag="mm", name=f"ps2_{h}_{e}_1_{bt}"
                    )
                    wv = wb8.rearrange("p (dp i j) -> p dp i j", i=2, j=P)
                    for dp in range(ND // 2):
                        nc.tensor.matmul(
                            ps2,
                            wv[:, dp],
                            h1f8[dp].rearrange(
                                "p (b i n) -> p b i n", i=2, n=NT
                            )[:, bt],
                            start=(dp == 0),
                            stop=(dp == ND // 2 - 1),
                            perf_mode=DR,
                        )
                    flush_l3()
                    # psum = 64*16*z2; xa = 16*relu(z2+b22) = relu(ps/64+16b)
                    xa = xap.tile(
                        [P, NT], F8, tag="xa8", name=f"xa_{h}_{e}_1_{bt}"
                    )
                    nc.scalar.activation(
                        xa, ps2, AF.Relu, bias=b22t16[:, e:e + 1],
                        scale=1.0 / W8SC,
                    )
                    defer_l3(1, e, bt, w32t, xa)

                # ---- layer 1 interleaved with L2(e=0) ----
                # h1 is produced batch-tile by batch-tile; as soon as a batch
                # tile is complete, the e=0/br=0 L2 group for it runs. This
                # keeps PE dense across the head boundary (no HAM throttle).
                wb21_0 = load_wblock(W21_e, 0, "21", "v")
                wb22_0 = load_wblock8(0)

                h1 = [
                    h1p.tile([P, BL], BF16, tag=f"h1_{dc}", name=f"h1_{h}_{dc}")
                    for dc in range(ND)
                ]
                # fp8 copy of h1 (x16) in DoubleRow pair layout:
                # h1f8[dp][p, bt, i, n] = 16*h1[d = dp*256+i*128+p, bt*NT+n]
                h1f8 = [
                    h1p.tile([P, NB * 2 * NT], F8, tag=f"h1f8_{dp}",
                             name=f"h1f8_{h}_{dp}")
                    for dp in range(ND // 2)
                ]
                for bt in range(NB):
                    for dc in range(ND):
                        ps1 = psp.tile([P, NT], F32, tag="mm",
                                       name=f"ps1_{h}_{dc}_{bt}")
                        nc.tensor.matmul(
                            ps1,
                            w1tb[:, dc * P:(dc + 1) * P],
                            xTb[:, bt * NT:(bt + 1) * NT],
                            start=True,
                            stop=True,
                        )
                        h1s = h1[dc][:, bt * NT:(bt + 1) * NT]
                        # bf16 copy on DVE: (z + bias) max zeros
                        nc.vector.scalar_tensor_tensor(
                            h1s, ps1, b1t[:, dc:dc + 1], zNT,
                            op0=ALU.add, op1=ALU.max,
                        )
                        # fp8 x16 copy on ACT
                        h8s = h1f8[dc >> 1].rearrange(
                            "p (b i n) -> p b i n", i=2, n=NT
                        )[:, bt, dc & 1, :]
                        nc.scalar.activation(
                            h8s, ps1, AF.Relu, bias=b1t16[:, dc:dc + 1],
                            scale=A8SC,
                        )
                    l2_group_b1(0, bt, wb21_0)
                # previous head's QP epilogue lands here: after this head's
                # L1 acts are queued, so the act engines never stall PE at
                # the head boundary.
                while pending_epi:
                    pending_epi.pop(0)()
                for bt in range(NB):
                    l2_group_b2(0, bt, wb22_0)

                for e in range(1, NE):
                    wb21 = load_wblock(W21_e, e, "21", "v")
                    wb22 = load_wblock8(e)
                    for bt in range(NB):
                        l2_group_b1(e, bt, wb21)
                    for bt in range(NB):
                        l2_group_b2(e, bt, wb22)
                flush_l3()

                # ---- QP epilogue (deferred into the next head) ----
                # Runs entirely in grid space [128, Q]: the four psum rows
                # (x31/z32 x channel) are copied to SBUF, scattered to [Q, P]
                # via sbuf-sbuf DMA, PE-transposed to grids, then the QP math
                # is partition-parallel (Q=16-wide ops instead of BL-wide).
                def emit_epilogue(h=h, acc31=acc31, acc32=acc32):
                    t31f = ep.tile([P, NT], F32, tag="t31f",
                                   name=f"t31f_{h}", bufs=2)
                    nc.vector.tensor_copy(t31f, acc31)
                    t32f = ep.tile([P, NT], F32, tag="t32f",
                                   name=f"t32f_{h}", bufs=2)
                    nc.vector.tensor_copy(t32f, acc32)

                    g = {}
                    for br, tf in ((0, t31f), (1, t32f)):
                        for c in range(C):
                            og = ep.tile([Q, P], F32, tag="og",
                                         name=f"og_{h}_{br}_{c}", bufs=4)
                            for bt in range(NB):
                                nc.sync.dma_start(
                                    out=og[4 * bt:4 * bt + 4, :],
                                    in_=tf[32 * bt + c:32 * bt + c + 1, :]
                                    .rearrange("one (q p) -> one q p", p=P),
                                )
                            tp = psp.tile([P, Q], F32, tag="mm",
                                          name=f"tpz_{h}_{br}_{c}")
                            nc.tensor.matmul(tp, og, ident[0:Q, 0:Q],
                                             is_transpose=True,
                                             start=True, stop=True)
                            zg = ep.tile([P, Q], F32, tag="zg",
                                         name=f"zg_{h}_{br}_{c}", bufs=8)
                            nc.scalar.copy(zg, tp)
                            g[(br, c)] = zg

                    def eg(nm):
                        return ep.tile([P, Q], F32, tag="eg",
                                       name=f"{nm}_{h}", bufs=10)

                    # acc32 = 64*16*z32 -> sigmoid(z32 + b32) via scale
                    s0, s1 = eg("s0"), eg("s1")
                    nc.scalar.activation(
                        s0, g[(1, 0)], AF.Sigmoid,
                        bias=B32B[:, h * C:h * C + 1],
                        scale=1.0 / (W8SC * A8SC),
                    )
                    nc.scalar.activation(
                        s1, g[(1, 1)], AF.Sigmoid,
                        bias=B32B[:, h * C + 1:h * C + 2],
                        scale=1.0 / (W8SC * A8SC),
                    )
                    x310, x311 = eg("x310"), eg("x311")
                    nc.vector.tensor_scalar(
                        x310, g[(0, 0)], B31B[:, h * C:h * C + 1], None,
                        op0=ALU.add,
                    )
                    nc.vector.tensor_scalar(
                        x311, g[(0, 1)], B31B[:, h * C + 1:h * C + 2], None,
                        op0=ALU.add,
                    )

                    # h_rhs = Lf2b + ssum*bdot4 + sprod*bar16
                    ssum, sprod = eg("ssum"), eg("sprod")
                    nc.vector.tensor_add(ssum, s0, s1)
                    nc.vector.tensor_mul(sprod, s0, s1)
                    nc.vector.tensor_mul(ssum, ssum, bdot4g)
                    nc.vector.tensor_mul(sprod, sprod, bar16g)
                    nc.vector.tensor_add(ssum, ssum, sprod)
                    hrhs = eg("hrhs")
                    nc.vector.tensor_add(hrhs, ssum, Lf2bg)

                    # lam = relu(G.x31 - hrhs) * invGG
                    gu0, gu1 = eg("gu0"), eg("gu1")
                    nc.vector.tensor_mul(gu0, G0g, x310)
                    nc.vector.tensor_mul(gu1, G1g, x311)
                    nc.vector.tensor_add(gu0, gu0, gu1)
                    nc.vector.tensor_sub(gu0, gu0, hrhs)
                    nc.vector.tensor_scalar_max(gu0, gu0, 0.0)
                    lam = eg("lam")
                    nc.vector.tensor_mul(lam, gu0, invGGg)

                    # u_c = x31_c - lam*G_c ; outacc_c += w[h]*u_c
                    lg0, lg1 = eg("lg0"), eg("lg1")
                    nc.vector.tensor_mul(lg0, lam, G0g)
                    nc.vector.tensor_sub(x310, x310, lg0)
                    nc.vector.scalar_tensor_tensor(
                        outacc0g, x310, wB[:, h:h + 1], outacc0g,
                        op0=ALU.mult, op1=ALU.add,
                    )
                    nc.vector.tensor_mul(lg1, lam, G1g)
                    nc.vector.tensor_sub(x311, x311, lg1)
                    nc.vector.scalar_tensor_tensor(
                        outacc1g, x311, wB[:, h:h + 1], outacc1g,
                        op0=ALU.mult, op1=ALU.add,
                    )

                pending_epi.append(emit_epilogue)

            while pending_epi:
                pending_epi.pop(0)()

            # ---------------- output ---------------------------------------
            # outacc grids -> [128, 16x2] interleave, one near-contiguous DMA
            # (8-byte segments) instead of 4-byte scatters.
            outT = ep.tile([P, Q * C], F32, tag="outT", name="outT", bufs=1)
            ov = outT.rearrange("p (q c) -> p c q", c=C)
            nc.scalar.copy(ov[:, 0, :], outacc0g)
            nc.scalar.copy(ov[:, 1, :], outacc1g)
            nc.sync.dma_start(
                out=out_e.rearrange("(q p) c -> p q c", p=P),
                in_=outT.rearrange("p (q c) -> p q c", c=C),
            )

    nc.finalize()
    return nc


_nc_cache = None


def _get_nc():
    global _nc_cache
    if _nc_cache is None:
        _nc_cache = build_nc()
    return _nc_cache


_WEIGHT_NAMES = (
    "W1", "b1", "W21", "b21", "W22", "b22",
    "W31", "b31", "W32", "b32", "wt", "mean", "std",
)


_BF16_NAMES = ("W1", "W21", "W31")
_F8_NAMES = ("W22", "W32")


def kernel(**inputs) -> np.ndarray:
    import ml_dtypes

    x = np.ascontiguousarray(np.asarray(inputs["x"], dtype=np.float32))
    rep = {}
    for k in _WEIGHT_NAMES:
        a = np.asarray(inputs[k], dtype=np.float32)
        if k in _BF16_NAMES:
            a = a.astype(ml_dtypes.bfloat16)
        elif k in _F8_NAMES:
            a = (a * W8SC).astype(ml_dtypes.float8_e4m3)
        rep[k] = np.ascontiguousarray(a)
    nc = _get_nc()
    in_maps = []
    for i in range(N_CORES):
        m = dict(rep)
        m["x"] = np.ascontiguousarray(x[i * BL_FULL:(i + 1) * BL_FULL])
        in_maps.append(m)
    globals()["_last_in_maps"] = in_maps
    res = run_bass_kernel_spmd(nc, in_maps, core_ids=list(range(N_CORES)))
    outs = [np.asarray(res.results[i]["out"]) for i in range(N_CORES)]
    return np.concatenate(outs, axis=0).astype(np.float32)



# revision 29
# speedup vs baseline: 1.3855x; 1.1539x over previous
"""Trainium2 Bass kernel for the ABNet 10-head MLP ensemble + dCBF QP problem.

Sharding: pure data-parallel over the batch axis (B=16384 -> 2048 per core,
8 cores). All per-sample math, including the closed-form 1-constraint QP, is
local to a core; weights are replicated; no collectives.

Per-core compute layout (feature-major, batch in the free dimension):
  xT   [4, BL]        x transposed  (moving operand of layer 1)
  h1   [2048, BL]     = relu(W1.T x) stored as 16 chunks [128, BL] bf16
  L2   x2b[e,b]       = relu(sum_d W2b[d,e] h1[d,b]) via PE, psum [128, 512]
  L3   z3b[c,b]       = sum_e W3b[e,c] x2b[e,b], accumulated in psum at
                        partition offset 32*bt (PE array tiling)
  QP epilogue on DVE/ACT in fp32 on [1, 512] rows, weighted head sum.

Matmuls run in bf16 (1 cycle/row on PE vs 4 for fp32) with fp32 PSUM
accumulation; all non-matmul math stays fp32.

Branch 2 (W22 -> x22 -> x32 -> sigmoid CBF params) runs in fp8e4 with
DoubleRow perf mode (2 contraction rows per PE cycle, ~1.8x measured):
the sigmoid + QP structure fully absorbs fp8 quantization error
(measured end-to-end rel-err identical to all-bf16). Branch 1 (x31,
the control path) must stay bf16 (fp8 there fails the 2e-2 gate).
Scales: W22/W32 pre-scaled x64 on host; h1/x22 activations x16 on
device; descale folded into the next activation's scale operand.
"""

import numpy as np

import concourse.bass as bass
import concourse.bacc as bacc
import concourse.mybir as mybir
from concourse.tile import TileContext
from concourse.bass_utils import run_bass_kernel_spmd
from concourse.masks import make_identity

F32 = mybir.dt.float32
BF16 = mybir.dt.bfloat16
F8 = mybir.dt.float8e4
DR = mybir.MatmulPerfMode.DoubleRow
AF = mybir.ActivationFunctionType
ALU = mybir.AluOpType
AX = mybir.AxisListType

W8SC = 64.0   # host-side fp8 weight scale (W22, W32)
A8SC = 16.0   # on-device fp8 activation scale (h1, x22)

OBS_X, OBS_Y, RADIUS = 40.0, 15.0, 6.0
PI = float(np.pi)
TWO_PI = 2.0 * PI

N_CORES = 8
H_FULL, B_FULL, F_FULL, D_FULL, C_FULL = 10, 16384, 4, 2048, 2
BL_FULL = B_FULL // N_CORES

P = 128


def build_nc(H=H_FULL, F=F_FULL, D=D_FULL, C=C_FULL, BL=BL_FULL, NT=512):
    """Build the single-core Bass graph (SPMD: same graph on all cores)."""
    ND = D // P          # contraction chunks (layer 2)
    NE = D // P          # output-feature chunks (layer 2) == L3 contraction
    NB = BL // NT        # batch tiles
    Q = BL // P          # grid columns (sample b = q*128 + p)
    assert D % P == 0 and BL % NT == 0 and NB <= 4 and BL % P == 0

    nc = bacc.Bacc(None, target_bir_lowering=False)

    x_e = nc.declare_dram_parameter("x", [BL, F], F32, isOutput=False)
    W1_e = nc.declare_dram_parameter("W1", [H, F, D], BF16, isOutput=False)
    b1_e = nc.declare_dram_parameter("b1", [H, D], F32, isOutput=False)
    W21_e = nc.declare_dram_parameter("W21", [H, D, D], BF16, isOutput=False)
    b21_e = nc.declare_dram_parameter("b21", [H, D], F32, isOutput=False)
    W22_e = nc.declare_dram_parameter("W22", [H, D, D], F8, isOutput=False)
    b22_e = nc.declare_dram_parameter("b22", [H, D], F32, isOutput=False)
    W31_e = nc.declare_dram_parameter("W31", [H, D, C], BF16, isOutput=False)
    b31_e = nc.declare_dram_parameter("b31", [H, C], F32, isOutput=False)
    W32_e = nc.declare_dram_parameter("W32", [H, D, C], F8, isOutput=False)
    b32_e = nc.declare_dram_parameter("b32", [H, C], F32, isOutput=False)
    wt_e = nc.declare_dram_parameter("wt", [H], F32, isOutput=False)
    mean_e = nc.declare_dram_parameter("mean", [F], F32, isOutput=False)
    std_e = nc.declare_dram_parameter("std", [F], F32, isOutput=False)
    out_e = nc.declare_dram_parameter("out", [BL, C], F32, isOutput=True)

    with (
        TileContext(nc) as tc,
        tc.tile_pool(name="cp", bufs=1) as cp,
        tc.tile_pool(name="ps", bufs=4, space="PSUM") as psp,
        tc.tile_pool(name="accp", bufs=2, space="PSUM") as accp,
    ):
        # persistent per-sample rows + small constants
        def crow(tagname):
            return cp.tile([1, BL], F32, tag=tagname, name=tagname)

        xTb = cp.tile([F, BL], BF16, tag="xTb", name="xTb")
        # QP constraint vectors and output accumulators live in GRID form
        # [128, Q] (sample b = q*128 + p at [p, q]) — partition-parallel
        # epilogue math and only 64B/partition each (vs 8KB for [1,BL] rows)
        def cgrid(nm):
            return cp.tile([P, Q], F32, tag=nm, name=nm)

        bar16g, bdot4g, Lf2bg = cgrid("bar16g"), cgrid("bdot4g"), cgrid("Lf2bg")
        G0g, G1g, invGGg = cgrid("G0g"), cgrid("G1g"), cgrid("invGGg")
        outacc0g, outacc1g = cgrid("outacc0g"), cgrid("outacc1g")
        wrow = cp.tile([1, H], F32, tag="wrow", name="wrow")
        # per-head scalars broadcast to all 128 partitions (grid-math biases)
        wB = cp.tile([P, H], F32, tag="wB", name="wB")
        B31B = cp.tile([P, H * C], F32, tag="B31B", name="B31B")
        B32B = cp.tile([P, H * C], F32, tag="B32B", name="B32B")

        # identity for PE transposes
        ident = cp.tile([P, P], F32, tag="ident", name="ident")
        make_identity(nc, ident)

        # ~100us of light serial DVE work before anything that gates the
        # dense phase: starting the kernel at full blast latches the chip
        # into the 2.0 GHz power state; a gentle ramp keeps it at 2.4.
        warm = cp.tile([1, NT], F32, tag="warm", name="warm")
        nc.vector.memset(warm, 0.0)
        for _ in range(192):
            nc.vector.tensor_scalar(warm, warm, 1.0, None, op0=ALU.add)
        # gate: dummy write into xTb (immediately overwritten by the real
        # producer; exists only to order the dense phase after the ramp)
        nc.vector.tensor_copy(xTb[0:1, 0:1], warm[0:1, 0:1])

        # ------------- preamble (scratch pool, freed afterwards) -----------
        # Per-sample math runs partition-parallel on [128, 16] "grid" tiles
        # (sample b = q*128 + p lives at [p, q]); the six QP vectors the
        # epilogue needs are then transposed back to [1, BL] rows via PE.
        with tc.tile_pool(name="pre", bufs=1) as pre:
            xload = pre.tile([P, Q * F], F32, tag="xload", name="xload")
            nc.sync.dma_start(
                out=xload.rearrange("p (q f) -> p q f", f=F),
                in_=x_e.rearrange("(q p) f -> p q f", p=P),
            )
            xg = xload.rearrange("p (q f) -> p f q", f=F)

            # broadcast std/mean to every partition with a ones-matmul
            smR = pre.tile([1, 2 * F], F32, tag="smR", name="smR")
            nc.sync.dma_start(out=smR[:, 0:F], in_=std_e[None, :])
            nc.sync.dma_start(out=smR[:, F:2 * F], in_=mean_e[None, :])
            ones1 = pre.tile([1, P], F32, tag="ones1", name="ones1")
            nc.vector.memset(ones1, 1.0)
            psb = psp.tile([P, 2 * F], F32, tag="mm", name="ps_bcast")
            nc.tensor.matmul(psb, ones1, smR, start=True, stop=True)
            smB = pre.tile([P, 2 * F], F32, tag="smB", name="smB")
            nc.scalar.copy(smB, psb)

            def grid(nm):
                return pre.tile([P, Q], F32, tag=nm, name=nm)

            x0g = []
            for f in range(F):
                t = grid(f"x0g{f}")
                nc.vector.tensor_scalar(t, xg[:, f, :], smB[:, f:f + 1], None,
                                        op0=ALU.mult)
                nc.vector.tensor_scalar(t, t, smB[:, F + f:F + f + 1], None,
                                        op0=ALU.add)
                x0g.append(t)
            pxg, pyg, thg, vg = x0g

            # sin with range reduction into [-pi, pi] (|arg| < 5*pi)
            def sin_reduced(out_t, arg_ap, sa, sb):
                nc.vector.tensor_scalar(sa, arg_ap, 0.0, None, op0=ALU.add)
                for _ in range(2):
                    nc.vector.tensor_scalar(sb, sa, PI, None, op0=ALU.is_gt)
                    nc.vector.scalar_tensor_tensor(
                        sa, sb, -TWO_PI, sa, op0=ALU.mult, op1=ALU.add
                    )
                    nc.vector.tensor_scalar(sb, sa, -PI, None, op0=ALU.is_lt)
                    nc.vector.scalar_tensor_tensor(
                        sa, sb, TWO_PI, sa, op0=ALU.mult, op1=ALU.add
                    )
                nc.scalar.activation(out_t, sa, AF.Sin)

            sa, sb = grid("sa"), grid("sb")
            st, ct = grid("st"), grid("ct")
            sin_reduced(st, thg, sa, sb)
            thc = grid("thc")
            nc.vector.tensor_scalar(thc, thg, PI / 2.0, None, op0=ALU.add)
            sin_reduced(ct, thc, sa, sb)

            dxg, dyg = grid("dxg"), grid("dyg")
            nc.vector.tensor_scalar(dxg, pxg, -OBS_X, None, op0=ALU.add)
            nc.vector.tensor_scalar(dyg, pyg, -OBS_Y, None, op0=ALU.add)
            vstg, vctg = grid("vstg"), grid("vctg")
            nc.vector.tensor_mul(vstg, vg, st)
            nc.vector.tensor_mul(vctg, vg, ct)

            # bar16 = 16*(dx^2 + dy^2 - R^2)
            nc.vector.tensor_mul(sa, dxg, dxg)
            nc.vector.tensor_mul(sb, dyg, dyg)
            nc.vector.tensor_add(sa, sa, sb)
            nc.vector.tensor_scalar(
                bar16g, sa, -(RADIUS * RADIUS), 16.0, op0=ALU.add, op1=ALU.mult
            )
            # bdot4 = 8*(dx*vct + dy*vst)
            nc.vector.tensor_mul(sa, dxg, vctg)
            nc.vector.tensor_mul(sb, dyg, vstg)
            nc.vector.tensor_add(sa, sa, sb)
            nc.vector.tensor_scalar(bdot4g, sa, 8.0, None, op0=ALU.mult)
            # Lf2b = 2*v^2
            nc.scalar.activation(Lf2bg, vg, AF.Square, scale=float(np.sqrt(2.0)))
            # G0 = 2*(dx*vst - dy*vct); G1 = -2*(dx*ct + dy*st)
            nc.vector.tensor_mul(sa, dxg, vstg)
            nc.vector.tensor_mul(sb, dyg, vctg)
            nc.vector.tensor_sub(sa, sa, sb)
            nc.vector.tensor_scalar(G0g, sa, 2.0, None, op0=ALU.mult)
            nc.vector.tensor_mul(sa, dxg, ct)
            nc.vector.tensor_mul(sb, dyg, st)
            nc.vector.tensor_add(sa, sa, sb)
            nc.vector.tensor_scalar(G1g, sa, -2.0, None, op0=ALU.mult)
            nc.vector.tensor_mul(sa, G0g, G0g)
            nc.vector.tensor_mul(sb, G1g, G1g)
            nc.vector.tensor_add(sa, sa, sb)
            nc.vector.reciprocal(invGGg, sa)

            # convert the raw x grids -> xTb rows (PE transpose + DMA)
            def grid_to_row(gt, row_ap, dtype, nm):
                tp = psp.tile([Q, P], F32, tag="mm", name=f"tp_{nm}")
                nc.tensor.matmul(tp, gt, ident, is_transpose=True,
                                 start=True, stop=True)
                cvt = pre.tile([Q, P], dtype, tag="cvt" + dtype.name,
                               name=f"cvt_{nm}", bufs=2)
                nc.scalar.copy(cvt, tp)
                nc.sync.dma_start(
                    out=row_ap.rearrange("one (q p) -> one q p", p=P),
                    in_=cvt,
                )

            for f in range(F):
                grid_to_row(xg[:, f, :], xTb[f:f + 1, :], BF16, f"xtb{f}")

            # broadcast b31/b32 (all heads) to every partition: [P, H*C]
            b3R = pre.tile([1, 2 * H * C], F32, tag="b3R", name="b3R")
            nc.sync.dma_start(
                out=b3R[:, 0:H * C].rearrange("one (h c) -> one h c", c=C),
                in_=b31_e[None, :, :],
            )
            nc.sync.dma_start(
                out=b3R[:, H * C:].rearrange("one (h c) -> one h c", c=C),
                in_=b32_e[None, :, :],
            )
            psb3 = psp.tile([P, 2 * H * C], F32, tag="mm", name="ps_b3")
            nc.tensor.matmul(psb3, ones1, b3R, start=True, stop=True)
            nc.scalar.copy(B31B, psb3[:, 0:H * C])
            nc.scalar.copy(B32B, psb3[:, H * C:])

            # softmax over wt -> wrow [1, H]
            wt_row = pre.tile([1, H], F32, tag="wt_row", name="wt_row")
            nc.sync.dma_start(out=wt_row, in_=wt_e[None, :])
            wred = pre.tile([1, 1], F32, tag="wred", name="wred")
            nc.vector.reduce_max(wred, wt_row, axis=AX.X)
            nwmax = pre.tile([1, 1], F32, tag="nwmax", name="nwmax")
            nc.vector.tensor_scalar(nwmax, wred, -1.0, None, op0=ALU.mult)
            wexp = pre.tile([1, H], F32, tag="wexp", name="wexp")
            nc.scalar.activation(wexp, wt_row, AF.Exp, bias=nwmax)
            nc.vector.reduce_sum(wred, wexp, axis=AX.X)
            winv = pre.tile([1, 1], F32, tag="winv", name="winv")
            nc.vector.reciprocal(winv, wred)
            nc.vector.tensor_scalar(wrow, wexp, winv, None, op0=ALU.mult)
            psw = psp.tile([P, H], F32, tag="mm", name="ps_w")
            nc.tensor.matmul(psw, ones1, wrow, start=True, stop=True)
            nc.scalar.copy(wB, psw)

            nc.vector.memset(outacc0g, 0.0)
            nc.vector.memset(outacc1g, 0.0)

        # ------------- main pools + head loop ------------------------------
        with (
            tc.tile_pool(name="hw", bufs=2) as hp,      # per-head small tensors
            tc.tile_pool(name="wb", bufs=4) as wbp,     # bf16 weight blocks
            tc.tile_pool(name="h1p", bufs=1) as h1p,
            tc.tile_pool(name="xap", bufs=5) as xap,
            tc.tile_pool(name="ep", bufs=8) as ep,     # epilogue scratch
        ):
            zNT = cp.tile([P, NT], BF16, tag="zNT", name="zNT")
            nc.vector.memset(zNT, 0.0)

            pending_epi = []
            pending_l3 = []

            def flush_l3():
                while pending_l3:
                    pending_l3.pop(0)()

            def head_smalls(h):
                sm = {"h": h}
                w1tb = hp.tile([F, D], BF16, tag="w1tb", name=f"w1tb_{h}")
                nc.sync.dma_start(out=w1tb, in_=W1_e[h])
                b1t = hp.tile([P, ND], F32, tag="b1t", name=f"b1t_{h}")
                nc.sync.dma_start(
                    out=b1t, in_=b1_e[h].rearrange("(dc p) -> p dc", p=P)
                )
                b21t = hp.tile([P, NE], F32, tag="b21t", name=f"b21t_{h}")
                nc.sync.dma_start(
                    out=b21t, in_=b21_e[h].rearrange("(ec p) -> p ec", p=P)
                )
                b22t = hp.tile([P, NE], F32, tag="b22t", name=f"b22t_{h}")
                nc.sync.dma_start(
                    out=b22t, in_=b22_e[h].rearrange("(ec p) -> p ec", p=P)
                )
                # relu(16z + 16b) = 16 relu(z + b): x16 bias for fp8 acts
                b22t16 = hp.tile([P, NE], F32, tag="b22t16", name=f"b22t16_{h}")
                nc.vector.tensor_scalar(b22t16, b22t, A8SC, None, op0=ALU.mult)
                w31t = hp.tile([P, NE * C], BF16, tag="w31t", name=f"w31t_{h}")
                nc.sync.dma_start(
                    out=w31t.rearrange("p (ec c) -> p ec c", c=C),
                    in_=W31_e[h].rearrange("(ec p) c -> p ec c", p=P),
                )
                w32t = hp.tile([P, NE * C], F8, tag="w32t", name=f"w32t_{h}")
                nc.sync.dma_start(
                    out=w32t.rearrange("p (ec c) -> p ec c", c=C),
                    in_=W32_e[h].rearrange("(ec p) c -> p ec c", p=P),
                )
                sm.update(w1tb=w1tb, b1t=b1t, b21t=b21t, b22t16=b22t16,
                          w31t=w31t, w32t=w32t)
                return sm

            def make_l1_steps(sm):
                # 64 (matmul + DVE relu) closures; interleaved into the
                # previous head's branch2 phase so the relu drain never
                # stalls PE (the relus are ~3x slower than L1 matmuls).
                h = sm["h"]
                h1 = [
                    h1p.tile([P, BL], BF16, tag=f"h1_{dc}", name=f"h1_{h}_{dc}")
                    for dc in range(ND)
                ]
                sm["h1"] = h1
                steps = []
                for bt in range(NB):
                    for dc in range(ND):
                        def step(bt=bt, dc=dc):
                            ps1 = psp.tile([P, NT], F32, tag="mm",
                                           name=f"ps1_{h}_{dc}_{bt}")
                            nc.tensor.matmul(
                                ps1,
                                sm["w1tb"][:, dc * P:(dc + 1) * P],
                                xTb[:, bt * NT:(bt + 1) * NT],
                                start=True,
                                stop=True,
                            )
                            h1s = h1[dc][:, bt * NT:(bt + 1) * NT]
                            nc.vector.scalar_tensor_tensor(
                                h1s, ps1, sm["b1t"][:, dc:dc + 1], zNT,
                                op0=ALU.add, op1=ALU.max,
                            )
                        steps.append(step)
                return steps

            def load_wblock(h, e):
                wb = wbp.tile([P, D], BF16, tag="wb", name=f"wb21_{h}_{e}",
                              bufs=2)
                nc.sync.dma_start(
                    out=wb.rearrange("p (dc j) -> p dc j", j=P),
                    in_=W21_e[h][:, e * P:(e + 1) * P].rearrange(
                        "(dc p) j -> p dc j", p=P
                    ),
                )
                return wb

            def load_wblock8(h, e):
                # fp8 W22 e-block in DoubleRow pair layout:
                # wb[p, dp, i, j] = 64*W22[dp*256 + i*128 + p, e*128 + j]
                wb = wbp.tile([P, D], F8, tag="wb8", name=f"wb8_{h}_{e}",
                              bufs=2)
                nc.sync.dma_start(
                    out=wb.rearrange("p (dp i j) -> p dp i j", i=2, j=P),
                    in_=W22_e[h][:, e * P:(e + 1) * P].rearrange(
                        "(dp i p) j -> p dp i j", i=2, p=P
                    ),
                )
                return wb

            def branch1_phase(h, sm):
                # Per-branch psum accumulator; batch-tile lanes live at
                # partition offsets 0/32/64/96 (PE-array tile positions).
                acc31 = accp.tile([128, NT], F32, tag="acc31",
                                  name=f"acc31_{h}")
                sm["acc31"] = acc31
                h1 = sm["h1"]

                # fp8 copy of h1 (x16) in DoubleRow pair layout, produced by
                # cheap ACT sbuf->sbuf copies spread over this phase (ACT is
                # otherwise idle here):
                # h1f8[dp][p, bt, i, n] = 16*h1[d = dp*256+i*128+p, bt*NT+n]
                h1f8 = [
                    h1p.tile([P, NB * 2 * NT], F8, tag=f"h1f8_{dp}",
                             name=f"h1f8_{h}_{dp}")
                    for dp in range(ND // 2)
                ]
                sm["h1f8"] = h1f8
                copies = [(bt, dc) for bt in range(NB) for dc in range(ND)]
                ci = [0]

                def emit_copies(k):
                    while k > 0 and ci[0] < len(copies):
                        bt, dc = copies[ci[0]]
                        ci[0] += 1
                        k -= 1
                        h8s = h1f8[dc >> 1].rearrange(
                            "p (b i n) -> p b i n", i=2, n=NT
                        )[:, bt, dc & 1, :]
                        nc.scalar.mul(
                            h8s, h1[dc][:, bt * NT:(bt + 1) * NT], A8SC
                        )

                def l2_group_b1(e, bt, wb):
                    ps2 = psp.tile(
                        [P, NT], F32, tag="mm", name=f"ps2_{h}_{e}_0_{bt}"
                    )
                    for dc in range(ND):
                        nc.tensor.matmul(
                            ps2,
                            wb[:, dc * P:(dc + 1) * P],
                            h1[dc][:, bt * NT:(bt + 1) * NT],
                            start=(dc == 0),
                            stop=(dc == ND - 1),
                        )
                    if len(pending_l3) >= NB:
                        flush_l3()
                    xa = xap.tile(
                        [P, NT], BF16, tag="xa", name=f"xa_{h}_{e}_0_{bt}",
                        bufs=6,
                    )
                    # relu(z+b) on DVE
                    nc.vector.scalar_tensor_tensor(
                        xa, ps2, sm["b21t"][:, e:e + 1], zNT,
                        op0=ALU.add, op1=ALU.max,
                    )
                    sl = 32 * bt

                    def emit_l3():
                        nc.tensor.matmul(
                            acc31[sl:sl + 2, :],
                            sm["w31t"][:, C * e:C * (e + 1)],
                            xa,
                            start=(e == 0),
                            stop=(e == NE - 1),
                            skip_group_check=True,
                            tile_position=(0, sl),
                        )

                    pending_l3.append(emit_l3)

                for e in range(NE):
                    wb21 = load_wblock(h, e)
                    for bt in range(NB):
                        l2_group_b1(e, bt, wb21)
                    emit_copies(4)
                    if e == 0:
                        # previous head's QP epilogue (grid-space, cheap)
                        while pending_epi:
                            pending_epi.pop(0)()
                emit_copies(len(copies))

            def branch2_phase(h, sm, interleave):
                acc32 = accp.tile([128, NT], F32, tag="acc32",
                                  name=f"acc32_{h}")
                sm["acc32"] = acc32
                h1f8 = sm["h1f8"]

                def l2_group_b2(e, bt, wb8):
                    ps2 = psp.tile(
                        [P, NT], F32, tag="mm", name=f"ps2_{h}_{e}_1_{bt}"
                    )
                    wv = wb8.rearrange("p (dp i j) -> p dp i j", i=2, j=P)
                    for dp in range(ND // 2):
                        nc.tensor.matmul(
                            ps2,
                            wv[:, dp],
                            h1f8[dp].rearrange(
                                "p (b i n) -> p b i n", i=2, n=NT
                            )[:, bt],
                            start=(dp == 0),
                            stop=(dp == ND // 2 - 1),
                            perf_mode=DR,
                        )
                    if len(pending_l3) >= NB:
                        flush_l3()
                    # psum = 64*16*z2; xa = 16*relu(z2+b22) = relu(ps/64+16b)
                    xa = xap.tile(
                        [P, NT], F8, tag="xa8", name=f"xa_{h}_{e}_1_{bt}",
                        bufs=6,
                    )
                    nc.scalar.activation(
                        xa, ps2, AF.Relu, bias=sm["b22t16"][:, e:e + 1],
                        scale=1.0 / W8SC,
                    )
                    sl = 32 * bt

                    def emit_l3():
                        nc.tensor.matmul(
                            acc32[sl:sl + 2, :],
                            sm["w32t"][:, C * e:C * (e + 1)],
                            xa,
                            start=(e == 0),
                            stop=(e == NE - 1),
                            skip_group_check=True,
                            tile_position=(0, sl),
                        )

                    pending_l3.append(emit_l3)

                for e in range(NE):
                    wb22 = load_wblock8(h, e)
                    for bt in range(NB):
                        l2_group_b2(e, bt, wb22)
                        if interleave:
                            interleave.pop(0)()
                flush_l3()

            # ---- software pipeline over heads ----
            # ---- QP epilogue (deferred into the next head's b1 phase) ----
            # Runs entirely in grid space [128, Q]: the four psum rows
            # (x31/z32 x channel) are copied to SBUF, scattered to [Q, P]
            # via sbuf-sbuf DMA, PE-transposed to grids, then the QP math
            # is partition-parallel (Q=16-wide ops instead of BL-wide).
            def make_epilogue(h, sm):
                acc31, acc32 = sm["acc31"], sm["acc32"]

                def emit_epilogue():
                    t31f = ep.tile([P, NT], F32, tag="t31f",
                                   name=f"t31f_{h}", bufs=2)
                    nc.vector.tensor_copy(t31f, acc31)
                    t32f = ep.tile([P, NT], F32, tag="t32f",
                                   name=f"t32f_{h}", bufs=2)
                    nc.vector.tensor_copy(t32f, acc32)

                    g = {}
                    for br, tf in ((0, t31f), (1, t32f)):
                        for c in range(C):
                            og = ep.tile([Q, P], F32, tag="og",
                                         name=f"og_{h}_{br}_{c}", bufs=4)
                            for bt in range(NB):
                                nc.sync.dma_start(
                                    out=og[4 * bt:4 * bt + 4, :],
                                    in_=tf[32 * bt + c:32 * bt + c + 1, :]
                                    .rearrange("one (q p) -> one q p", p=P),
                                )
                            tp = psp.tile([P, Q], F32, tag="mm",
                                          name=f"tpz_{h}_{br}_{c}")
                            nc.tensor.matmul(tp, og, ident[0:Q, 0:Q],
                                             is_transpose=True,
                                             start=True, stop=True)
                            zg = ep.tile([P, Q], F32, tag="zg",
                                         name=f"zg_{h}_{br}_{c}", bufs=8)
                            nc.scalar.copy(zg, tp)
                            g[(br, c)] = zg

                    def eg(nm):
                        return ep.tile([P, Q], F32, tag="eg",
                                       name=f"{nm}_{h}", bufs=10)

                    # acc32 = 64*16*z32 -> sigmoid(z32 + b32) via scale
                    s0, s1 = eg("s0"), eg("s1")
                    nc.scalar.activation(
                        s0, g[(1, 0)], AF.Sigmoid,
                        bias=B32B[:, h * C:h * C + 1],
                        scale=1.0 / (W8SC * A8SC),
                    )
                    nc.scalar.activation(
                        s1, g[(1, 1)], AF.Sigmoid,
                        bias=B32B[:, h * C + 1:h * C + 2],
                        scale=1.0 / (W8SC * A8SC),
                    )
                    x310, x311 = eg("x310"), eg("x311")
                    nc.vector.tensor_scalar(
                        x310, g[(0, 0)], B31B[:, h * C:h * C + 1], None,
                        op0=ALU.add,
                    )
                    nc.vector.tensor_scalar(
                        x311, g[(0, 1)], B31B[:, h * C + 1:h * C + 2], None,
                        op0=ALU.add,
                    )

                    # h_rhs = Lf2b + ssum*bdot4 + sprod*bar16
                    ssum, sprod = eg("ssum"), eg("sprod")
                    nc.vector.tensor_add(ssum, s0, s1)
                    nc.vector.tensor_mul(sprod, s0, s1)
                    nc.vector.tensor_mul(ssum, ssum, bdot4g)
                    nc.vector.tensor_mul(sprod, sprod, bar16g)
                    nc.vector.tensor_add(ssum, ssum, sprod)
                    hrhs = eg("hrhs")
                    nc.vector.tensor_add(hrhs, ssum, Lf2bg)

                    # lam = relu(G.x31 - hrhs) * invGG
                    gu0, gu1 = eg("gu0"), eg("gu1")
                    nc.vector.tensor_mul(gu0, G0g, x310)
                    nc.vector.tensor_mul(gu1, G1g, x311)
                    nc.vector.tensor_add(gu0, gu0, gu1)
                    nc.vector.tensor_sub(gu0, gu0, hrhs)
                    nc.vector.tensor_scalar_max(gu0, gu0, 0.0)
                    lam = eg("lam")
                    nc.vector.tensor_mul(lam, gu0, invGGg)

                    # u_c = x31_c - lam*G_c ; outacc_c += w[h]*u_c
                    lg0, lg1 = eg("lg0"), eg("lg1")
                    nc.vector.tensor_mul(lg0, lam, G0g)
                    nc.vector.tensor_sub(x310, x310, lg0)
                    nc.vector.scalar_tensor_tensor(
                        outacc0g, x310, wB[:, h:h + 1], outacc0g,
                        op0=ALU.mult, op1=ALU.add,
                    )
                    nc.vector.tensor_mul(lg1, lam, G1g)
                    nc.vector.tensor_sub(x311, x311, lg1)
                    nc.vector.scalar_tensor_tensor(
                        outacc1g, x311, wB[:, h:h + 1], outacc1g,
                        op0=ALU.mult, op1=ALU.add,
                    )

                return emit_epilogue

            # ---- software pipeline over heads ----
            # [b1(h) | h1f8 copies(h) | epi(h-1)] [b2(h) | L1(h+1)]
            sm = head_smalls(0)
            for s in make_l1_steps(sm):
                s()
            for h in range(H):
                branch1_phase(h, sm)
                if h + 1 < H:
                    sm_next = head_smalls(h + 1)
                    nxt = make_l1_steps(sm_next)
                else:
                    sm_next, nxt = None, []
                branch2_phase(h, sm, nxt)
                for s in nxt:
                    s()
                pending_epi.append(make_epilogue(h, sm))
                sm = sm_next

            while pending_epi:
                pending_epi.pop(0)()

            # ---------------- output ---------------------------------------
            # outacc grids -> [128, 16x2] interleave, one near-contiguous DMA
            # (8-byte segments) instead of 4-byte scatters.
            outT = ep.tile([P, Q * C], F32, tag="outT", name="outT", bufs=1)
            ov = outT.rearrange("p (q c) -> p c q", c=C)
            nc.scalar.copy(ov[:, 0, :], outacc0g)
            nc.scalar.copy(ov[:, 1, :], outacc1g)
            nc.sync.dma_start(
                out=out_e.rearrange("(q p) c -> p q c", p=P),
                in_=outT.rearrange("p (q c) -> p q c", c=C),
            )

    nc.finalize()
    return nc


_nc_cache = None


def _get_nc():
    global _nc_cache
    if _nc_cache is None:
        _nc_cache = build_nc()
    return _nc_cache


_WEIGHT_NAMES = (
    "W1", "b1", "W21", "b21", "W22", "b22",
    "W31", "b31", "W32", "b32", "wt", "mean", "std",
)


_BF16_NAMES = ("W1", "W21", "W31")
_F8_NAMES = ("W22", "W32")


def kernel(**inputs) -> np.ndarray:
    import ml_dtypes

    x = np.ascontiguousarray(np.asarray(inputs["x"], dtype=np.float32))
    rep = {}
    for k in _WEIGHT_NAMES:
        a = np.asarray(inputs[k], dtype=np.float32)
        if k in _BF16_NAMES:
            a = a.astype(ml_dtypes.bfloat16)
        elif k in _F8_NAMES:
            a = (a * W8SC).astype(ml_dtypes.float8_e4m3)
        rep[k] = np.ascontiguousarray(a)
    nc = _get_nc()
    in_maps = []
    for i in range(N_CORES):
        m = dict(rep)
        m["x"] = np.ascontiguousarray(x[i * BL_FULL:(i + 1) * BL_FULL])
        in_maps.append(m)
    globals()["_last_in_maps"] = in_maps
    res = run_bass_kernel_spmd(nc, in_maps, core_ids=list(range(N_CORES)))
    outs = [np.asarray(res.results[i]["out"]) for i in range(N_CORES)]
    return np.concatenate(outs, axis=0).astype(np.float32)

